# revision 1
# baseline (speedup 1.0000x reference)
"""Trainium2 Bass kernel for a biaffine-style dependency-parser layer (DEPLayer).

Computes, for B=8 examples of T=128 tokens (D=400 in, H=300 hidden, L=45 labels):
    h[t,s,:]  = relu(a_proj[t] + b_proj[s] + b1)         (s over T+1 head candidates)
    arc[t,s]  = h[t,s,:] @ Wa                            (UAS logits)
    sel_h[t]  = h[t, desired_arcs[t], :]
    lab[t,:]  = sel_h[t] @ Wl                            (LAS logits)
    loss      = mean-masked CE(arc) / CE(lab) averaged

Sharding: data-parallel over batch across the 8 NeuronCores (1 example/core),
params replicated.  The device never materializes the [T,T+1,H] tensor: per
(s, H-chunk) one fused add+relu (VectorE tensor_scalar, bf16 4x mode) builds
the [H_chunk, T] tile relu(a_projT + b1 + BtT[:, s]) which the PE immediately
reduces with Wa into the arc psum column s (relu tile stationary, Wa moving,
so the output lands in the natural [T, S] layout; the narrow 44-row chunk
packs two s values per matmul via a block-diagonal Wa pair).  Final
softmax/CE and the scalar loss reduction happen on host in float64 (ba
shifts every arc logit equally so it cancels in log_softmax exactly; bl is
added on host).
"""

import numpy as np
from contextlib import ExitStack

import concourse.bacc as bacc
import concourse.bass as bass
import concourse.tile as tile
import concourse.mybir as mybir
from concourse.bass_utils import run_bass_kernel_spmd

B, T, D, H, L = 8, 128, 400, 300, 45
S = T + 1  # head candidates (root + T tokens)

F32 = mybir.dt.float32
BF16 = mybir.dt.bfloat16

# contraction (D) chunks and hidden (H) chunks, both limited to 128 partitions
DK = [(0, 128), (128, 128), (256, 128), (384, 16)]
HC = [(0, 128), (128, 128), (256, 44)]

_COMPILED = None  # cached (nc) — compile once per process

# relu-tile engine rotation (D=VectorE, A=ScalarE, P=GpSimdE), tuned via the
# instruction cost model; override with env BASSK_PATTERN for experiments
# All relu tiles go to VectorE: measured on HW, GpSimd tensor_scalar is far
# slower than its cost-model estimate and ScalarE sharing also loses to
# DVE-only (DVE runs tensor_scalar bf16 in 4x mode).
_ENGINE_PATTERN = list(
    __import__("os").environ.get("BASSK_PATTERN", "D")
)
_RT_BUFS = int(__import__("os").environ.get("BASSK_RTBUFS", "48"))


def _build_kernel():
    nc = bacc.Bacc(
        "TRN2",
        target_bir_lowering=False,
        debug=False,
        num_devices=B,
    )

    xrT = nc.dram_tensor("xrT", [D, S], BF16, kind="ExternalInput").ap()
    w1a = nc.dram_tensor("w1a", [D, H], BF16, kind="ExternalInput").ap()
    w1b = nc.dram_tensor("w1b", [D, H], BF16, kind="ExternalInput").ap()
    # packed small params: col 0 = b1, col 1 = Wa, cols 2:2+L = Wl
    prm = nc.dram_tensor("prm", [H, 2 + L], F32, kind="ExternalInput").ap()
    gt = nc.dram_tensor("gt", [S, T], BF16, kind="ExternalInput").ap()
    arc = nc.dram_tensor("arc", [T, S], F32, kind="ExternalOutput").ap()
    labT = nc.dram_tensor("labT", [L, T], F32, kind="ExternalOutput").ap()

    reps = int(__import__("os").environ.get("BASSK_REPS", "1"))
    with tile.TileContext(nc) as tc:
        for _ in range(reps):
            _kernel_body(tc, xrT, w1a, w1b, prm, gt, arc, labT)

    nc.compile()
    return nc


def _kernel_body(tc, xrT, w1a, w1b, prm, gt, arc, labT):
    nc = tc.nc
    with ExitStack() as ctx:
        consts = ctx.enter_context(tc.tile_pool(name="consts", bufs=1))
        work = ctx.enter_context(tc.tile_pool(name="work", bufs=1))
        rtp = ctx.enter_context(tc.tile_pool(name="rt", bufs=8))
        psum = ctx.enter_context(
            tc.tile_pool(name="psum", bufs=1, space=bass.MemorySpace.PSUM)
        )

        # ---- load replicated params + per-core activations into SBUF ----
        # issue DMAs round-robin across sequencers: one SP sequencer issuing
        # all of them serializes the kernel start by several microseconds
        _de = __import__("os").environ.get("BASSK_DMAENG", "sag")
        dma_engs = {"s": [nc.sync], "sa": [nc.sync, nc.scalar],
                    "sag": [nc.sync, nc.scalar, nc.gpsimd]}[_de]
        dma_i = 0

        def dma(out_ap, in_ap):
            nonlocal dma_i
            dma_engs[dma_i % len(dma_engs)].dma_start(out_ap, in_ap)
            dma_i += 1

        xrt_sb = []
        w1a_sb = []
        w1b_sb = []
        for ki, (d0, dsz) in enumerate(DK):
            t_x = consts.tile([dsz, S], BF16, tag=f"xrt{ki}")
            dma(t_x[:, :], xrT[d0 : d0 + dsz, :])
            xrt_sb.append(t_x)
            t_a = consts.tile([dsz, H], BF16, tag=f"w1a{ki}")
            dma(t_a[:, :], w1a[d0 : d0 + dsz, :])
            w1a_sb.append(t_a)
            t_b = consts.tile([dsz, H], BF16, tag=f"w1b{ki}")
            dma(t_b[:, :], w1b[d0 : d0 + dsz, :])
            w1b_sb.append(t_b)

        b1_sb = []
        wa_sb = []
        wl_sb = []
        for c, (h0, hsz) in enumerate(HC):
            t_prm = consts.tile([hsz, 2 + L], F32, tag=f"prm{c}")
            dma(t_prm[:, :], prm[h0 : h0 + hsz, :])
            b1_sb.append(t_prm[:, 0:1])
            wa_sb.append(t_prm[:, 1:2])
            wl_sb.append(t_prm[:, 2 : 2 + L])

        gt0 = consts.tile([128, T], BF16, tag="gt0")
        dma(gt0[:, :], gt[0:128, :])
        gt1 = consts.tile([1, T], BF16, tag="gt1")
        dma(gt1[:, :], gt[128:129, :])

        # ---- b_projN = xr @ W1b as [128,H] + [1,H] (no b1)  ----
        pbn0 = psum.tile([128, H], F32, tag="pbn0", bufs=1)
        pbn1 = psum.tile([1, H], F32, tag="pbn1", bufs=1)
        for ki, (d0, dsz) in enumerate(DK):
            nc.tensor.matmul(
                pbn0[:, :],
                xrt_sb[ki][:, 0:128],
                w1b_sb[ki][:, :],
                start=(ki == 0),
                stop=(ki == len(DK) - 1),
            )
        for ki, (d0, dsz) in enumerate(DK):
            nc.tensor.matmul(
                pbn1[:, :],
                xrt_sb[ki][:, 128:129],
                w1b_sb[ki][:, :],
                start=(ki == 0),
                stop=(ki == len(DK) - 1),
            )
        bn0_sb = work.tile([128, H], BF16, tag="bn0")
        nc.vector.tensor_copy(bn0_sb[:, :], pbn0[:, :])
        bn1_sb = work.tile([1, H], BF16, tag="bn1")
        nc.vector.tensor_copy(bn1_sb[:, :], pbn1[:, :])

        # ---- per H-chunk: BtT = (xr @ W1b)^T  and  abias = (x @ W1a)^T + b1;
        #      sel_hT = relu(a_projT + (G @ b_projN)^T + b1)  reusing the psum ----
        btT_sb = []
        abias_sb = []
        selh_sb = []
        for c, (h0, hsz) in enumerate(HC):
            pbt = psum.tile([hsz, S], F32, tag="pbt", bufs=1)
            for ki, (d0, dsz) in enumerate(DK):
                nc.tensor.matmul(
                    pbt[:, :],
                    w1b_sb[ki][:, h0 : h0 + hsz],
                    xrt_sb[ki][:, :],
                    start=(ki == 0),
                    stop=(ki == len(DK) - 1),
                )
            t_bt = work.tile([hsz, S], F32, tag=f"btT{c}")
            nc.vector.tensor_copy(t_bt[:, :], pbt[:, :])
            btT_sb.append(t_bt)

            # a_projT chunk -> abias = a_projT + b1, in bf16 (arc-loop
            # add+relu input; bf16 enables DVE 4x and PE FWL fast load)
            pst = psum.tile([hsz, T], F32, tag="pselT", bufs=1)
            for ki, (d0, dsz) in enumerate(DK):
                nc.tensor.matmul(
                    pst[:, :],
                    w1a_sb[ki][:, h0 : h0 + hsz],
                    xrt_sb[ki][:, 1:S],
                    start=(ki == 0),
                    stop=(ki == len(DK) - 1),
                )
            t_ab = work.tile([hsz, T], BF16, tag=f"abias{c}")
            nc.scalar.activation(
                t_ab[:, :],
                pst[:, :],
                mybir.ActivationFunctionType.Identity,
                bias=b1_sb[c][:, 0:1],
            )
            abias_sb.append(t_ab)

            # sel_hT = relu(a_projT + (G @ b_projN)^T + b1); its own psum
            # group redoes the cheap a_projT matmuls
            ps2 = psum.tile([hsz, T], F32, tag="pselT2", bufs=1)
            for ki, (d0, dsz) in enumerate(DK):
                nc.tensor.matmul(
                    ps2[:, :],
                    w1a_sb[ki][:, h0 : h0 + hsz],
                    xrt_sb[ki][:, 1:S],
                    start=(ki == 0),
                    stop=False,
                )
            nc.tensor.matmul(
                ps2[:, :], bn0_sb[:, h0 : h0 + hsz], gt0[:, :], start=False, stop=False
            )
            nc.tensor.matmul(
                ps2[:, :], bn1_sb[:, h0 : h0 + hsz], gt1[:, :], start=False, stop=True
            )
            t_sh = work.tile([hsz, T], F32, tag=f"selh{c}")
            nc.scalar.activation(
                t_sh[:, :],
                ps2[:, :],
                mybir.ActivationFunctionType.Relu,
                bias=b1_sb[c][:, 0:1],
            )
            selh_sb.append(t_sh)

        # ---- label logits^T = Wl^T @ sel_h^T : [L, T] ----
        plab = psum.tile([L, T], F32, tag="plab", bufs=1)
        for c, (h0, hsz) in enumerate(HC):
            nc.tensor.matmul(
                plab[:, :],
                wl_sb[c][:, :],
                selh_sb[c][:, :],
                start=(c == 0),
                stop=(c == len(HC) - 1),
            )
        labT_sb = work.tile([L, T], F32, tag="labT")
        nc.vector.tensor_copy(labT_sb[:, :], plab[:, :])
        nc.sync.dma_start(labT[:, :], labT_sb[:, :])

        # ---- main pairwise loop (s-major): arc[t, s] = Wa . relu(abias[:,t]
        #      + BtT[:,s]).  Per (s, chunk): one fused add+relu -> bf16 tile
        #      [hsz, T], then a PE matmul with the tile *stationary* (128
        #      bf16 weight columns -> FWL fast load) and Wa moving, emitting
        #      the natural [T, 1] psum column of arc. ----
        wab_sb = []
        for c, (h0, hsz) in enumerate(HC):
            t_wab = consts.tile([hsz, 1], BF16, tag=f"wab{c}")
            nc.vector.tensor_copy(t_wab[:, :], wa_sb[c][:, :])
            wab_sb.append(t_wab)

        # ---- pairing setup for the narrow 44-partition chunk (c=2): stack
        #      two s-values on partitions [0:44]+[44:88] so one relu instr
        #      and one matmul (block-diagonal Wa pair) cover both ----
        # engine ops need 32-aligned start partitions: stack the second s at
        # offset 64 and zero the unused stripe (its Wa rows are zero too)
        h2, hsz2 = HC[2]
        OFF2 = 64
        P2 = OFF2 + hsz2  # 108
        npairs = (S - 1) // 2  # 64 pairs cover s=0..127; s=128 is a tail
        abias2x = work.tile([P2, T], BF16, tag="abias2x")
        nc.vector.memset(abias2x[:, :], 0.0)
        nc.vector.tensor_copy(abias2x[0:hsz2, :], abias_sb[2][:, :])
        nc.vector.tensor_copy(abias2x[OFF2:P2, :], abias_sb[2][:, :])
        bt2x = work.tile([P2, npairs], F32, tag="bt2x")
        nc.vector.memset(bt2x[:, :], 0.0)
        nc.vector.tensor_copy(bt2x[0:hsz2, :], btT_sb[2][:, 0 : 2 * npairs : 2])
        nc.vector.tensor_copy(bt2x[OFF2:P2, :], btT_sb[2][:, 1 : 2 * npairs : 2])
        wa_pair = work.tile([P2, 2], BF16, tag="wa_pair")
        nc.vector.memset(wa_pair[:, :], 0.0)
        nc.vector.tensor_copy(wa_pair[0:hsz2, 0:1], wa_sb[2][:, :])
        nc.vector.tensor_copy(wa_pair[OFF2:P2, 1:2], wa_sb[2][:, :])

        # manual tile rings (avoids per-iteration pool alloc/release instrs)
        rings = {0: [], 1: [], 2: []}
        ring_it = {0: 0, 1: 0, 2: 0}

        def ring_tile(kind):
            lst = rings[kind]
            r = ring_it[kind] % _RT_BUFS
            ring_it[kind] += 1
            while len(lst) <= r:
                part = 128 if kind < 2 else P2
                lst.append(
                    rtp.tile(
                        [part, T],
                        BF16,
                        name=f"ring{kind}_{len(lst)}",
                        tag=f"ring{kind}_{len(lst)}",
                        bufs=1,
                    )
                )
            return lst[r]

        PATTERN = _ENGINE_PATTERN
        NOPE = __import__("os").environ.get("BASSK_NOPE", "0") == "1"
        NORELU = __import__("os").environ.get("BASSK_NORELU", "0") == "1"
        COLSPLIT = __import__("os").environ.get("BASSK_COLSPLIT", "1") == "1"
        idx = 0

        HALVES = ((0, 64), (64, T))

        def arc_col(out_fn, tiles):
            # tiles: list of (lhsT_tile, psz, rhs_ap) accumulated into one
            # psum column region.  COLSPLIT runs the column as two 64-wide
            # col-group halves (sequential groups, so the second half's
            # LDWEIGHTS can overlap the first half's MATMULs on the PE).
            if not COLSPLIT:
                for i, (lt, psz, rhs_ap) in enumerate(tiles):
                    nc.tensor.matmul(
                        out_fn(0, T), lt[0:psz, :], rhs_ap,
                        start=(i == 0), stop=(i == len(tiles) - 1),
                    )
                return
            for t0, t1 in HALVES:
                for i, (lt, psz, rhs_ap) in enumerate(tiles):
                    nc.tensor.matmul(
                        out_fn(t0, t1), lt[0:psz, t0:t1], rhs_ap,
                        start=(i == 0), stop=(i == len(tiles) - 1),
                        tile_position=(0, t0),
                    )

        def emit_relu(rt_ap, in_ap, bias_ap):
            nonlocal idx
            eng = PATTERN[idx % len(PATTERN)]
            idx += 1
            if eng == "A":
                nc.scalar.activation(
                    rt_ap,
                    in_ap,
                    mybir.ActivationFunctionType.Relu,
                    bias=bias_ap,
                )
            else:
                veng = nc.vector if eng == "D" else nc.gpsimd
                veng.tensor_scalar(
                    rt_ap,
                    in_ap,
                    bias_ap,
                    0.0,
                    mybir.AluOpType.add,
                    mybir.AluOpType.max,
                )

        parc = psum.tile([T, S], F32, tag="parc", bufs=1)
        parc2 = None if NOPE else psum.tile([T, S - 1], F32, tag="parc2", bufs=1)
        for j in range(npairs):
            for jj in range(2):
                s = 2 * j + jj
                col_tiles = []
                for c in (0, 1):
                    rt = ring_tile(c)
                    if not NORELU or ring_it[c] <= _RT_BUFS:
                        emit_relu(rt[:, :], abias_sb[c][:, :], btT_sb[c][:, s : s + 1])
                    col_tiles.append((rt, 128, wab_sb[c][:, :]))
                if not NOPE:
                    arc_col(lambda t0, t1, s=s: parc[t0:t1, s : s + 1], col_tiles)
            rt2 = ring_tile(2)
            if not NORELU or ring_it[2] <= _RT_BUFS:
                emit_relu(rt2[:, :], abias2x[:, :], bt2x[:, j : j + 1])
            if not NOPE:
                arc_col(
                    lambda t0, t1, j=j: parc2[t0:t1, 2 * j : 2 * j + 2],
                    [(rt2, P2, wa_pair[:, :])],
                )
        # tail column s = S-1 (all three chunks accumulate in parc)
        s = S - 1
        tail_tiles = []
        for c, (h0, hsz) in enumerate(HC):
            rt = ring_tile(min(c, 2))
            if not NORELU:
                emit_relu(rt[0:hsz, :], abias_sb[c][:, :], btT_sb[c][:, s : s + 1])
            tail_tiles.append((rt, hsz, wab_sb[c][:, :]))
        arc_col(lambda t0, t1, s=s: parc[t0:t1, s : s + 1], tail_tiles)

        arc_sb = work.tile([T, S], F32, tag="arc")
        nc.vector.tensor_copy(arc_sb[:, :], parc[:, :])
        if not NOPE:
            nc.vector.tensor_tensor(
                arc_sb[:, 0 : S - 1],
                arc_sb[:, 0 : S - 1],
                parc2[:, :],
                mybir.AluOpType.add,
            )
        nc.sync.dma_start(arc[:, :], arc_sb[:, :])


def _get_compiled():
    global _COMPILED
    if _COMPILED is None:
        _COMPILED = _build_kernel()
    return _COMPILED


def _log_softmax64(x):
    x = x.astype(np.float64)
    m = x.max(axis=-1, keepdims=True)
    e = np.exp(x - m)
    return x - m - np.log(e.sum(axis=-1, keepdims=True))


def build_in_maps(inputs):
    import ml_dtypes

    bf16 = ml_dtypes.bfloat16
    cont = np.asarray(inputs["cont_repr"], np.float32)
    root = np.asarray(inputs["root"], np.float32).reshape(1, D)
    W1a = np.ascontiguousarray(np.asarray(inputs["W1a"], np.float32)).astype(bf16)
    W1b = np.ascontiguousarray(np.asarray(inputs["W1b"], np.float32)).astype(bf16)
    prm = np.concatenate(
        [
            np.asarray(inputs["b1"], np.float32).reshape(H, 1),
            np.asarray(inputs["Wa"], np.float32).reshape(H, 1),
            np.asarray(inputs["Wl"], np.float32).reshape(H, L),
        ],
        axis=1,
    )  # [H, 2+L]
    des = np.asarray(inputs["desired_arcs"]).astype(np.int64)

    in_maps = []
    for i in range(B):
        xr = np.concatenate([root, cont[i]], axis=0)  # [S, D]
        GT = (des[i][None, :] == np.arange(S)[:, None]).astype(bf16)  # [S,T]
        in_maps.append(
            {
                "xrT": np.ascontiguousarray(xr.T).astype(bf16),
                "w1a": W1a,
                "w1b": W1b,
                "prm": np.ascontiguousarray(prm),
                "gt": np.ascontiguousarray(GT),
            }
        )
    return in_maps


def run_device(inputs, trace=False):
    """Shard inputs, run the SPMD Bass kernel on 8 cores, return per-core
    (arc_logits [T,S], labT [L,T]) plus the BassKernelResults (for timing)."""
    in_maps = build_in_maps(inputs)
    nc = _get_compiled()
    res = run_bass_kernel_spmd(nc, in_maps, core_ids=list(range(B)), trace=trace)
    arcs = np.stack([res.results[i]["arc"] for i in range(B)])  # [B,T,S]
    labTs = np.stack([res.results[i]["labT"] for i in range(B)])  # [B,L,T]
    return arcs, labTs, res


def kernel(**inputs):
    arcs, labTs, _ = run_device(inputs)
    return _finalize(inputs, arcs, labTs)


def _finalize(inputs, arcs, labTs):
    lens = np.asarray(inputs["sentence_lengths"]).astype(np.int64)  # [B]
    des = np.asarray(inputs["desired_arcs"]).astype(np.int64)  # [B,T]
    lbls = np.asarray(inputs["desired_labels"]).astype(np.int64)  # [B,T]
    blv = np.asarray(inputs["bl"], np.float64)  # [L]
    use_des = bool(int(np.asarray(inputs["use_desired_arcs"])))

    mask = (np.arange(T)[None, :] < lens[:, None]).astype(np.float64)  # [B,T]
    n_valid = max(mask.sum(), 1.0)

    arc_logits = arcs.astype(np.float64)  # [B,T,S] (ba cancels in log_softmax)
    arc_lp = _log_softmax64(arc_logits)
    arc_ce = -np.take_along_axis(arc_lp, des[..., None], axis=-1)[..., 0]
    uas = (arc_ce * mask).sum() / n_valid

    if use_des:
        lab_logits = np.transpose(labTs, (0, 2, 1)).astype(np.float64) + blv
    else:
        # predicted-arcs branch: gather indices depend on the device arc
        # logits, so rebuild the (cheap) label path on host from them.
        pred = arc_logits.argmax(axis=-1)  # [B,T]
        cont = np.asarray(inputs["cont_repr"], np.float64)
        root = np.asarray(inputs["root"], np.float64).reshape(1, D)
        W1a = np.asarray(inputs["W1a"], np.float64)
        W1b = np.asarray(inputs["W1b"], np.float64)
        b1v = np.asarray(inputs["b1"], np.float64)
        Wlv = np.asarray(inputs["Wl"], np.float64)
        lab_logits = np.empty((B, T, L))
        for i in range(B):
            xr = np.concatenate([root, cont[i]], axis=0)  # [S,D]
            a_proj = cont[i] @ W1a  # [T,H]
            b_proj = xr @ W1b  # [S,H]
            sel_h = np.maximum(a_proj + b_proj[pred[i]] + b1v, 0.0)
            lab_logits[i] = sel_h @ Wlv + blv

    lab_lp = _log_softmax64(lab_logits)
    lab_ce = -np.take_along_axis(lab_lp, lbls[..., None], axis=-1)[..., 0]
    las = (lab_ce * mask).sum() / n_valid

    return np.float32((uas + las) / 2.0)



# revision 16
# speedup vs baseline: 1.4077x; 1.4077x over previous
"""Trainium2 Bass kernel for a biaffine-style dependency-parser layer (DEPLayer).

Computes, for B=8 examples of T=128 tokens (D=400 in, H=300 hidden, L=45 labels):
    h[t,s,:]  = relu(a_proj[t] + b_proj[s] + b1)         (s over T+1 head candidates)
    arc[t,s]  = h[t,s,:] @ Wa                            (UAS logits)
    sel_h[t]  = h[t, desired_arcs[t], :]
    lab[t,:]  = sel_h[t] @ Wl                            (LAS logits)
    loss      = mean-masked CE(arc) / CE(lab) averaged

Sharding: data-parallel over batch across the 8 NeuronCores (1 example/core),
params replicated.  The device never materializes the [T,T+1,H] tensor: per
(s, H-chunk) one fused add+relu (VectorE tensor_scalar, bf16 4x mode) builds
the [H_chunk, T] tile relu(a_projT + b1 + BtT[:, s]) which the PE immediately
reduces with Wa into the arc psum column s (relu tile stationary, Wa moving,
so the output lands in the natural [T, S] layout; the narrow 44-row chunk
packs two s values per matmul via a block-diagonal Wa pair).  Final
softmax/CE and the scalar loss reduction happen on host in float64 (ba
shifts every arc logit equally so it cancels in log_softmax exactly; bl is
added on host).
"""

import numpy as np
from contextlib import ExitStack

import concourse.bacc as bacc
import concourse.bass as bass
import concourse.tile as tile
import concourse.mybir as mybir
from concourse.bass_utils import run_bass_kernel_spmd

B, T, D, H, L = 8, 128, 400, 300, 45
S = T + 1  # head candidates (root + T tokens)

F32 = mybir.dt.float32
BF16 = mybir.dt.bfloat16

# contraction (D) chunks and hidden (H) chunks, both limited to 128 partitions
DK = [(0, 128), (128, 128), (256, 128), (384, 16)]
HC = [(0, 128), (128, 128), (256, 44)]

_COMPILED = None  # cached (nc) — compile once per process

# relu-tile engine rotation (D=VectorE, A=ScalarE, P=GpSimdE), tuned via the
# instruction cost model; override with env BASSK_PATTERN for experiments
# All relu tiles go to VectorE: measured on HW, GpSimd tensor_scalar is far
# slower than its cost-model estimate and ScalarE sharing also loses to
# DVE-only (DVE runs tensor_scalar bf16 in 4x mode).
_ENGINE_PATTERN = list(
    __import__("os").environ.get("BASSK_PATTERN", "DDA")
)

# effective token count: tokens t >= max(sentence_lengths) are masked out of
# the loss, so the device only computes t < TE.  Set from the actual inputs
# in run_device() before compiling (runtime JIT specialization); the s-axis
# (head candidates, incl. padding tokens) must stay S=T+1 because the arc
# softmax runs over all S candidates.
TE = T


def _te_from_inputs(inputs):
    import numpy as np

    lens = np.asarray(inputs["sentence_lengths"]).astype(np.int64)
    mx = int(lens.max()) if lens.size else T
    mx = max(4, min(T, mx))
    # round up to a multiple of 64: measured on HW, stationary tiles with
    # M not a multiple of 64 run the PE LDWEIGHTS/MATMUL ~2x slower, which
    # costs more than the trimmed token columns save
    return (mx + 63) // 64 * 64
_RT_BUFS = int(__import__("os").environ.get("BASSK_RTBUFS", "48"))


def _enable_ldw_opt():
    """The axon-precomputed neuronx-cc flag bundle ships
    --enable-ldw-opt=false; fast LDWEIGHTS measurably helps this kernel
    (per-s matvecs reload stationary weights constantly).  Flip it for our
    own compiles only."""
    if __import__("os").environ.get("BASSK_LDWOPT", "1") == "0":
        return
    try:
        from concourse.compiler_utils import get_compiler_flags, set_compiler_flags

        flags = get_compiler_flags()
        nf = [f.replace("--enable-ldw-opt=false", "--enable-ldw-opt=true") for f in flags]
        if nf != flags:
            set_compiler_flags(nf)
    except Exception:
        pass


def _build_kernel():
    _enable_ldw_opt()
    nc = bacc.Bacc(
        "TRN2",
        target_bir_lowering=False,
        debug=False,
        num_devices=B,
    )

    xrT = nc.dram_tensor("xrT", [D, S], BF16, kind="ExternalInput").ap()
    w1a = nc.dram_tensor("w1a", [D, H], BF16, kind="ExternalInput").ap()
    w1b = nc.dram_tensor("w1b", [D, H], BF16, kind="ExternalInput").ap()
    # packed small params: col 0 = b1, col 1 = Wa, cols 2:2+L = Wl
    prm = nc.dram_tensor("prm", [H, 2 + L], F32, kind="ExternalInput").ap()
    gt = nc.dram_tensor("gt", [S, TE], BF16, kind="ExternalInput").ap()
    arc = nc.dram_tensor("arc", [TE, S], F32, kind="ExternalOutput").ap()
    labT = nc.dram_tensor("labT", [L, TE], F32, kind="ExternalOutput").ap()

    reps = int(__import__("os").environ.get("BASSK_REPS", "1"))
    with tile.TileContext(nc) as tc:
        for _ in range(reps):
            _kernel_body(tc, xrT, w1a, w1b, prm, gt, arc, labT)

    nc.compile()
    return nc


def _kernel_body(tc, xrT, w1a, w1b, prm, gt, arc, labT):
    nc = tc.nc
    with ExitStack() as ctx:
        consts = ctx.enter_context(tc.tile_pool(name="consts", bufs=1))
        work = ctx.enter_context(tc.tile_pool(name="work", bufs=1))
        rtp = ctx.enter_context(tc.tile_pool(name="rt", bufs=8))
        # early-phase psum pool (b_proj / abias / sel_h / label logits):
        # closed before the arc accumulators so their 6 bank-interleaved
        # psum banks fit in the 8-bank budget
        early_ctx = ExitStack()
        psum = early_ctx.enter_context(
            tc.tile_pool(name="psum_early", bufs=1, space=bass.MemorySpace.PSUM)
        )

        # ---- load replicated params + per-core activations into SBUF ----
        # issue DMAs round-robin across sequencers: one SP sequencer issuing
        # all of them serializes the kernel start by several microseconds
        _de = __import__("os").environ.get("BASSK_DMAENG", "sag")
        dma_engs = {"s": [nc.sync], "sa": [nc.sync, nc.scalar],
                    "sag": [nc.sync, nc.scalar, nc.gpsimd]}[_de]
        dma_i = 0

        def dma(out_ap, in_ap):
            nonlocal dma_i
            dma_engs[dma_i % len(dma_engs)].dma_start(out_ap, in_ap)
            dma_i += 1

        xrt_sb = []
        w1a_sb = []
        w1b_sb = []
        for ki, (d0, dsz) in enumerate(DK):
            t_x = consts.tile([dsz, S], BF16, tag=f"xrt{ki}")
            dma(t_x[:, :], xrT[d0 : d0 + dsz, :])
            xrt_sb.append(t_x)
            t_a = consts.tile([dsz, H], BF16, tag=f"w1a{ki}")
            dma(t_a[:, :], w1a[d0 : d0 + dsz, :])
            w1a_sb.append(t_a)
            t_b = consts.tile([dsz, H], BF16, tag=f"w1b{ki}")
            dma(t_b[:, :], w1b[d0 : d0 + dsz, :])
            w1b_sb.append(t_b)

        b1_sb = []
        wa_sb = []
        wl_sb = []
        for c, (h0, hsz) in enumerate(HC):
            t_prm = consts.tile([hsz, 2 + L], F32, tag=f"prm{c}")
            dma(t_prm[:, :], prm[h0 : h0 + hsz, :])
            b1_sb.append(t_prm[:, 0:1])
            wa_sb.append(t_prm[:, 1:2])
            wl_sb.append(t_prm[:, 2 : 2 + L])

        gt0 = consts.tile([128, TE], BF16, tag="gt0")
        dma(gt0[:, :], gt[0:128, :])
        gt1 = consts.tile([1, TE], BF16, tag="gt1")
        dma(gt1[:, :], gt[128:129, :])

        # ---- b_projN = xr @ W1b as [128,H] + [1,H] (no b1)  ----
        pbn0 = psum.tile([128, H], F32, tag="pbn0", bufs=1)
        pbn1 = psum.tile([1, H], F32, tag="pbn1", bufs=1)
        for ki, (d0, dsz) in enumerate(DK):
            nc.tensor.matmul(
                pbn0[:, :],
                xrt_sb[ki][:, 0:128],
                w1b_sb[ki][:, :],
                start=(ki == 0),
                stop=(ki == len(DK) - 1),
            )
        for ki, (d0, dsz) in enumerate(DK):
            nc.tensor.matmul(
                pbn1[:, :],
                xrt_sb[ki][:, 128:129],
                w1b_sb[ki][:, :],
                start=(ki == 0),
                stop=(ki == len(DK) - 1),
            )
        bn0_sb = work.tile([128, H], BF16, tag="bn0")
        nc.vector.tensor_copy(bn0_sb[:, :], pbn0[:, :])
        bn1_sb = work.tile([1, H], BF16, tag="bn1")
        nc.vector.tensor_copy(bn1_sb[:, :], pbn1[:, :])

        # ---- per H-chunk: BtT = (xr @ W1b)^T  and  abias = (x @ W1a)^T + b1;
        #      sel_hT = relu(a_projT + (G @ b_projN)^T + b1)  reusing the psum ----
        btT_sb = []
        abias_sb = []
        selh_sb = []
        for c, (h0, hsz) in enumerate(HC):
            pbt = psum.tile([hsz, S], F32, tag="pbt", bufs=1)
            for ki, (d0, dsz) in enumerate(DK):
                nc.tensor.matmul(
                    pbt[:, :],
                    w1b_sb[ki][:, h0 : h0 + hsz],
                    xrt_sb[ki][:, :],
                    start=(ki == 0),
                    stop=(ki == len(DK) - 1),
                )
            t_bt = work.tile([hsz, S], F32, tag=f"btT{c}")
            nc.vector.tensor_copy(t_bt[:, :], pbt[:, :])
            btT_sb.append(t_bt)

            # a_projT chunk -> abias = a_projT + b1, in bf16 (arc-loop
            # add+relu input; bf16 enables DVE 4x and PE FWL fast load)
            pst = psum.tile([hsz, TE], F32, tag="pselT", bufs=1)
            for ki, (d0, dsz) in enumerate(DK):
                nc.tensor.matmul(
                    pst[:, :],
                    w1a_sb[ki][:, h0 : h0 + hsz],
                    xrt_sb[ki][:, 1 : 1 + TE],
                    start=(ki == 0),
                    stop=(ki == len(DK) - 1),
                )
            t_ab = work.tile([hsz, TE], BF16, tag=f"abias{c}")
            nc.scalar.activation(
                t_ab[:, :],
                pst[:, :],
                mybir.ActivationFunctionType.Identity,
                bias=b1_sb[c][:, 0:1],
            )
            abias_sb.append(t_ab)

            # sel_hT = relu(a_projT + (G @ b_projN)^T + b1); its own psum
            # group redoes the cheap a_projT matmuls
            ps2 = psum.tile([hsz, TE], F32, tag="pselT2", bufs=1)
            for ki, (d0, dsz) in enumerate(DK):
                nc.tensor.matmul(
                    ps2[:, :],
                    w1a_sb[ki][:, h0 : h0 + hsz],
                    xrt_sb[ki][:, 1 : 1 + TE],
                    start=(ki == 0),
                    stop=False,
                )
            nc.tensor.matmul(
                ps2[:, :], bn0_sb[:, h0 : h0 + hsz], gt0[:, :], start=False, stop=False
            )
            nc.tensor.matmul(
                ps2[:, :], bn1_sb[:, h0 : h0 + hsz], gt1[:, :], start=False, stop=True
            )
            t_sh = work.tile([hsz, TE], F32, tag=f"selh{c}")
            nc.scalar.activation(
                t_sh[:, :],
                ps2[:, :],
                mybir.ActivationFunctionType.Relu,
                bias=b1_sb[c][:, 0:1],
            )
            selh_sb.append(t_sh)

        # ---- label logits^T = Wl^T @ sel_h^T : [L, T] ----
        plab = psum.tile([L, TE], F32, tag="plab", bufs=1)
        for c, (h0, hsz) in enumerate(HC):
            nc.tensor.matmul(
                plab[:, :],
                wl_sb[c][:, :],
                selh_sb[c][:, :],
                start=(c == 0),
                stop=(c == len(HC) - 1),
            )
        labT_sb = work.tile([L, TE], F32, tag="labT")
        nc.vector.tensor_copy(labT_sb[:, :], plab[:, :])
        nc.sync.dma_start(labT[:, :], labT_sb[:, :])

        # ---- main pairwise loop (s-major): arc[t, s] = Wa . relu(abias[:,t]
        #      + BtT[:,s]).  Per (s, chunk): one fused add+relu -> bf16 tile
        #      [hsz, T], then a PE matmul with the tile *stationary* (128
        #      bf16 weight columns -> FWL fast load) and Wa moving, emitting
        #      the natural [T, 1] psum column of arc. ----
        early_ctx.close()
        psum2 = ctx.enter_context(
            tc.tile_pool(name="psum_arc", bufs=1, space=bass.MemorySpace.PSUM)
        )

        wab_sb = []
        for c, (h0, hsz) in enumerate(HC):
            t_wab = consts.tile([hsz, 1], BF16, tag=f"wab{c}")
            nc.vector.tensor_copy(t_wab[:, :], wa_sb[c][:, :])
            wab_sb.append(t_wab)

        # ---- pairing setup for the narrow 44-partition chunk (c=2): stack
        #      two s-values on partitions [0:44]+[44:88] so one relu instr
        #      and one matmul (block-diagonal Wa pair) cover both ----
        # engine ops need 32-aligned start partitions: stack the second s at
        # offset 64 and zero the unused stripe (its Wa rows are zero too)
        h2, hsz2 = HC[2]
        OFF2 = 64
        P2 = OFF2 + hsz2  # 108
        npairs = (S - 1) // 2  # 64 pairs cover s=0..127; s=128 is a tail
        abias2x = work.tile([P2, TE], BF16, tag="abias2x")
        nc.vector.memset(abias2x[:, :], 0.0)
        nc.vector.tensor_copy(abias2x[0:hsz2, :], abias_sb[2][:, :])
        nc.vector.tensor_copy(abias2x[OFF2:P2, :], abias_sb[2][:, :])
        bt2x = work.tile([P2, npairs], F32, tag="bt2x")
        nc.vector.memset(bt2x[:, :], 0.0)
        nc.vector.tensor_copy(bt2x[0:hsz2, :], btT_sb[2][:, 0 : 2 * npairs : 2])
        nc.vector.tensor_copy(bt2x[OFF2:P2, :], btT_sb[2][:, 1 : 2 * npairs : 2])
        wa_pair = work.tile([P2, 2], BF16, tag="wa_pair")
        nc.vector.memset(wa_pair[:, :], 0.0)
        nc.vector.tensor_copy(wa_pair[0:hsz2, 0:1], wa_sb[2][:, :])
        nc.vector.tensor_copy(wa_pair[OFF2:P2, 1:2], wa_sb[2][:, :])

        # manual tile rings (avoids per-iteration pool alloc/release instrs)
        rings = {0: [], 1: [], 2: []}
        ring_it = {0: 0, 1: 0, 2: 0}

        def ring_tile(kind):
            lst = rings[kind]
            r = ring_it[kind] % _RT_BUFS
            ring_it[kind] += 1
            while len(lst) <= r:
                part = 128 if kind < 2 else P2
                lst.append(
                    rtp.tile(
                        [part, TE],
                        BF16,
                        name=f"ring{kind}_{len(lst)}",
                        tag=f"ring{kind}_{len(lst)}",
                        bufs=1,
                    )
                )
            return lst[r]

        PATTERN = _ENGINE_PATTERN
        NOPE = __import__("os").environ.get("BASSK_NOPE", "0") == "1"
        NORELU = __import__("os").environ.get("BASSK_NORELU", "0") == "1"
        COLSPLIT = __import__("os").environ.get("BASSK_COLSPLIT", "0") == "1"
        idx = 0

        HALVES = ((0, TE // 2), (TE // 2, TE))

        def arc_col(out_fn, tiles):
            # tiles: list of (lhsT_tile, psz, rhs_ap) accumulated into one
            # psum column region.  COLSPLIT runs the column as two 64-wide
            # col-group halves (sequential groups, so the second half's
            # LDWEIGHTS can overlap the first half's MATMULs on the PE).
            if not COLSPLIT:
                for i, (lt, psz, rhs_ap) in enumerate(tiles):
                    nc.tensor.matmul(
                        out_fn(0, TE), lt[0:psz, :], rhs_ap,
                        start=(i == 0), stop=(i == len(tiles) - 1),
                    )
                return
            for t0, t1 in HALVES:
                for i, (lt, psz, rhs_ap) in enumerate(tiles):
                    nc.tensor.matmul(
                        out_fn(t0, t1), lt[0:psz, t0:t1], rhs_ap,
                        start=(i == 0), stop=(i == len(tiles) - 1),
                        tile_position=(0, t0),
                    )

        def emit_relu(rt_ap, in_ap, bias_ap):
            nonlocal idx
            eng = PATTERN[idx % len(PATTERN)]
            idx += 1
            if eng == "A":
                nc.scalar.activation(
                    rt_ap,
                    in_ap,
                    mybir.ActivationFunctionType.Relu,
                    bias=bias_ap,
                )
            else:
                veng = nc.vector if eng == "D" else nc.gpsimd
                veng.tensor_scalar(
                    rt_ap,
                    in_ap,
                    bias_ap,
                    0.0,
                    mybir.AluOpType.add,
                    mybir.AluOpType.max,
                )

        # arc accumulators, padded so consecutive s land in different psum
        # banks (bank = 512 f32): col s -> (s%4)*512 + s//4 spreads the 322
        # single-column matmuls over 4 banks; pair j -> (j%2)*512 + (j//2)*2
        # over 2 more.  Avoids back-to-back accumulate turnaround on one bank.
        parc = psum2.tile([TE, 2048], F32, tag="parc", bufs=1)
        parc2 = None if NOPE else psum2.tile([TE, 1024], F32, tag="parc2", bufs=1)

        def pc_off(s):
            return (s % 4) * 512 + (s // 4)

        def p2_off(j):
            return (j % 2) * 512 + (j // 2) * 2
        for j in range(npairs):
            for jj in range(2):
                s = 2 * j + jj
                col_tiles = []
                for c in (0, 1):
                    rt = ring_tile(c)
                    if not NORELU or ring_it[c] <= _RT_BUFS:
                        emit_relu(rt[:, :], abias_sb[c][:, :], btT_sb[c][:, s : s + 1])
                    col_tiles.append((rt, 128, wab_sb[c][:, :]))
                if not NOPE:
                    arc_col(lambda t0, t1, o=pc_off(s): parc[t0:t1, o : o + 1], col_tiles)
            rt2 = ring_tile(2)
            if not NORELU or ring_it[2] <= _RT_BUFS:
                emit_relu(rt2[:, :], abias2x[:, :], bt2x[:, j : j + 1])
            if not NOPE:
                arc_col(
                    lambda t0, t1, o=p2_off(j): parc2[t0:t1, o : o + 2],
                    [(rt2, P2, wa_pair[:, :])],
                )
        # tail column s = S-1 (all three chunks accumulate in parc)
        s = S - 1
        tail_tiles = []
        for c, (h0, hsz) in enumerate(HC):
            rt = ring_tile(min(c, 2))
            if not NORELU:
                emit_relu(rt[0:hsz, :], abias_sb[c][:, :], btT_sb[c][:, s : s + 1])
            tail_tiles.append((rt, hsz, wab_sb[c][:, :]))
        arc_col(lambda t0, t1, o=pc_off(s): parc[t0:t1, o : o + 1], tail_tiles)

        arc_sb = work.tile([TE, S], F32, tag="arc")
        for k in range(4):
            ncols = (S - k + 3) // 4  # s = k, k+4, ... < S
            nc.vector.tensor_copy(
                arc_sb[:, k : S : 4], parc[:, k * 512 : k * 512 + ncols]
            )
        if not NOPE:
            for p in range(2):
                for r in range(2):
                    nc.vector.tensor_tensor(
                        arc_sb[:, 2 * p + r : S - 1 : 4],
                        arc_sb[:, 2 * p + r : S - 1 : 4],
                        parc2[:, p * 512 + r : p * 512 + 64 + r : 2],
                        mybir.AluOpType.add,
                    )
        nc.sync.dma_start(arc[:, :], arc_sb[:, :])


def _get_compiled():
    global _COMPILED
    if _COMPILED is None or _COMPILED[0] != TE:
        _COMPILED = (TE, _build_kernel())
    return _COMPILED[1]


def _log_softmax64(x):
    x = x.astype(np.float64)
    m = x.max(axis=-1, keepdims=True)
    e = np.exp(x - m)
    return x - m - np.log(e.sum(axis=-1, keepdims=True))


def build_in_maps(inputs):
    import ml_dtypes

    bf16 = ml_dtypes.bfloat16
    cont = np.asarray(inputs["cont_repr"], np.float32)
    root = np.asarray(inputs["root"], np.float32).reshape(1, D)
    W1a = np.ascontiguousarray(np.asarray(inputs["W1a"], np.float32)).astype(bf16)
    W1b = np.ascontiguousarray(np.asarray(inputs["W1b"], np.float32)).astype(bf16)
    prm = np.concatenate(
        [
            np.asarray(inputs["b1"], np.float32).reshape(H, 1),
            np.asarray(inputs["Wa"], np.float32).reshape(H, 1),
            np.asarray(inputs["Wl"], np.float32).reshape(H, L),
        ],
        axis=1,
    )  # [H, 2+L]
    des = np.asarray(inputs["desired_arcs"]).astype(np.int64)

    in_maps = []
    for i in range(B):
        xr = np.concatenate([root, cont[i]], axis=0)  # [S, D]
        GT = (des[i][None, :TE] == np.arange(S)[:, None]).astype(bf16)  # [S,TE]
        in_maps.append(
            {
                "xrT": np.ascontiguousarray(xr.T).astype(bf16),
                "w1a": W1a,
                "w1b": W1b,
                "prm": np.ascontiguousarray(prm),
                "gt": np.ascontiguousarray(GT),
            }
        )
    return in_maps


def run_device(inputs, trace=False):
    """Shard inputs, run the SPMD Bass kernel on 8 cores, return per-core
    (arc_logits [T,S], labT [L,T]) plus the BassKernelResults (for timing)."""
    global TE
    TE = _te_from_inputs(inputs)
    in_maps = build_in_maps(inputs)
    nc = _get_compiled()
    res = run_bass_kernel_spmd(nc, in_maps, core_ids=list(range(B)), trace=trace)
    # pad t >= TE back to the full token axis with zeros (those tokens are
    # masked out of the loss; zero logits keep the host softmax NaN-free)
    arcs = np.zeros((B, T, S), np.float32)
    labTs = np.zeros((B, L, T), np.float32)
    for i in range(B):
        arcs[i, :TE] = res.results[i]["arc"]
        labTs[i, :, :TE] = res.results[i]["labT"]
    return arcs, labTs, res


def kernel(**inputs):
    arcs, labTs, _ = run_device(inputs)
    return _finalize(inputs, arcs, labTs)


def _finalize(inputs, arcs, labTs):
    lens = np.asarray(inputs["sentence_lengths"]).astype(np.int64)  # [B]
    des = np.asarray(inputs["desired_arcs"]).astype(np.int64)  # [B,T]
    lbls = np.asarray(inputs["desired_labels"]).astype(np.int64)  # [B,T]
    blv = np.asarray(inputs["bl"], np.float64)  # [L]
    use_des = bool(int(np.asarray(inputs["use_desired_arcs"])))

    mask = (np.arange(T)[None, :] < lens[:, None]).astype(np.float64)  # [B,T]
    n_valid = max(mask.sum(), 1.0)

    arc_logits = arcs.astype(np.float64)  # [B,T,S] (ba cancels in log_softmax)
    arc_lp = _log_softmax64(arc_logits)
    arc_ce = -np.take_along_axis(arc_lp, des[..., None], axis=-1)[..., 0]
    uas = (arc_ce * mask).sum() / n_valid

    if use_des:
        lab_logits = np.transpose(labTs, (0, 2, 1)).astype(np.float64) + blv
    else:
        # predicted-arcs branch: gather indices depend on the device arc
        # logits, so rebuild the (cheap) label path on host from them.
        pred = arc_logits.argmax(axis=-1)  # [B,T]
        cont = np.asarray(inputs["cont_repr"], np.float64)
        root = np.asarray(inputs["root"], np.float64).reshape(1, D)
        W1a = np.asarray(inputs["W1a"], np.float64)
        W1b = np.asarray(inputs["W1b"], np.float64)
        b1v = np.asarray(inputs["b1"], np.float64)
        Wlv = np.asarray(inputs["Wl"], np.float64)
        lab_logits = np.empty((B, T, L))
        for i in range(B):
            xr = np.concatenate([root, cont[i]], axis=0)  # [S,D]
            a_proj = cont[i] @ W1a  # [T,H]
            b_proj = xr @ W1b  # [S,H]
            sel_h = np.maximum(a_proj + b_proj[pred[i]] + b1v, 0.0)
            lab_logits[i] = sel_h @ Wlv + blv

    lab_lp = _log_softmax64(lab_logits)
    lab_ce = -np.take_along_axis(lab_lp, lbls[..., None], axis=-1)[..., 0]
    las = (lab_ce * mask).sum() / n_valid

    return np.float32((uas + las) / 2.0)

